# revision 43
# baseline (speedup 1.0000x reference)
"""Trainium2 Bass kernel: CenterHeadIoU 1x1-conv stack (v2: PE quadrant packing).

Computes, for x = ct_feat [B=32, C=128, N=8192]:
  y = relu(bn(sh_w @ x))                       [B, 64, N]
  z_h = relu(bn_h(head_w1[h] @ y)), h=0..5     [B, 64, N] each
  out = concat_h(head_final_w[h] @ z_h) + b    [B, 12, N]

Sharding: data-parallel over batch, 4 batches per core on 8 cores; weights
replicated. BN folded into weights on host; matmul operands bf16. The final
per-channel bias b3 is added on the HOST after the gather (frees a PSUM
epilogue op per group).

v2 exploits PE sub-array concurrency (tile_position): matmuls whose
(row-group, col-group) rectangles are disjoint execute CONCURRENTLY
(HW-probed: 4-quadrant K=64/M=64 packs at 61ns/MM vs 216 serial; K=128/M=64
column pairs at 127ns/MM). Per 2-tile pair, 7 rounds of ~512 PE cycles:

  r1: mm1(u')  @cols 0-63   || mm3[gR]  @cols 64-127
  r2: QUAD mm2-p0: 4x [K=64,M=64] in the 4 array quadrants
  r3: mm3[gL]  @cols 0-63   || mm1(u'+1) @cols 64-127
  r4: QUAD mm2-p1
  r5: mm3[gL]               || mm3[gR]
  r6: QUAD mm2-p2
  r7: mm3[gL]               || mm3[gR]

mm1 writes the pair's pre-y into ONE psum bank vertically (tile u rows 0-63
@col0, tile u+1 rows 64-127 @col64), so act1 is one [128,512] ACT op per
pair. mm2 runs as 4 quadrant halves per p (y(u) feeds row-pos 0, y(u+1)
row-pos 64, via a duplicated stationary) writing one [128,1024] psum tile.
mm3 accumulates 4 tiles x 3 p-blocks (12 streams, K=128 M=64, live cols
12s..12s+12) into a po half-bank; EVEN groups use cols/partitions 0-63, ODD
64-127, so two groups pipeline in ONE bank and pair up in rounds r5/r7.
Group g's streams run at pairs 2g+1..2g+4, one tile per pair. Output is
DMA'd straight from PSUM (12-partition stripes), bias added on host.

PSUM: py 1 + pz 3x2 + po 1 = 8 banks exactly.

Elementwise (z eviction = relu+bias+bf16, psum->sbuf) is spread over the
only two PSUM-capable engines plus GPSIMD via a staging bounce:
  ACT: act1 [128,512] + z-p1 [128,1024]          (~1.68us/pair)
  DVE: z-p0 [128,1024] tensor_scalar(max -b, add b)  (~1.24us/pair)
  GPS: z-p2 2x [128,512] from an SBUF f32 stage (PSUM->SBUF DMA bounce;
       GPSIMD has no PSUM port)                   (~1.68us/pair)
relu(z+b) == max(z,-b)+b lets tensor_scalar apply the bias without ACT.

The PE starts clock-gated at 1.2 GHz (HAM) and reaches 2.4 GHz only after
~3.4us of sustained activity; the prologue front-loads latency-ordered DMAs
and pre-warms the PE with dummy matmuls on a memset scratch tile.

A post-pass moves multi-wait sync conditions onto single-wait NoOp carriers
(this walrus build caps sync waits per instruction).
"""

import os
import sys
import numpy as np

B, C_IN, N, HC = 32, 128, 8192, 64
NCORES = 8
BC = B // NCORES            # batches per core
F = 512                     # free-dim tile = one fp32 PSUM bank
NT = N // F                 # tiles per batch
T = BC * NT                 # tiles per core
NPAIR = T // 2              # 2-tile pairs per core
GS = 4                      # tiles per mm3 output group
NG = T // GS                # groups per core
EPS = 1e-5
HEAD_OUT = [3, 2, 1, 3, 2, 1]        # hm, reg, height, dim, rot, iou
PAIR_OFF = [0, 5, 9]                 # channel offset of pair p in the 12-ch out

_CACHE = {}
LAST_RESULTS = None
LAST_EXEC_NS = None


def _build_program():
    import concourse.bass as bass
    import concourse.mybir as mybir
    import concourse.tile as tile

    f32 = mybir.dt.float32
    bf16 = mybir.dt.bfloat16
    AF = mybir.ActivationFunctionType
    ALU = mybir.AluOpType

    nc = bass.Bass("TRN2", target_bir_lowering=False, debug=False,
                   num_devices=NCORES)

    x = nc.dram_tensor("x", [BC, C_IN, N], bf16, kind="ExternalInput").ap()
    w1 = nc.dram_tensor("w1", [C_IN, HC], bf16, kind="ExternalInput").ap()
    b1d = nc.dram_tensor("b1d", [128, 1], f32, kind="ExternalInput").ap()
    w2d = nc.dram_tensor("w2d", [128, 384], bf16, kind="ExternalInput").ap()
    b2n = nc.dram_tensor("b2n", [128, 3], f32, kind="ExternalInput").ap()
    b2p = nc.dram_tensor("b2p", [128, 3], f32, kind="ExternalInput").ap()
    w3s = nc.dram_tensor("w3s", [128, 3 * GS * 64], bf16,
                         kind="ExternalInput").ap()
    out = nc.dram_tensor("out", [BC, 12, N], f32, kind="ExternalOutput").ap()

    with tile.TileContext(nc) as tc:
        with (
            tc.tile_pool(name="consts", bufs=1) as cpool,
            tc.tile_pool(name="xin", bufs=7) as xpool,
            tc.tile_pool(name="ysb", bufs=4) as ypool,
            tc.tile_pool(name="z0sb", bufs=5) as z0pool,
            tc.tile_pool(name="z1sb", bufs=5) as z1pool,
            tc.tile_pool(name="z2sb", bufs=5) as z2pool,
            tc.tile_pool(name="zst", bufs=4) as stpool,
            tc.tile_pool(name="ppy", bufs=1, space="PSUM") as pypool,
            tc.tile_pool(name="pz0", bufs=1, space="PSUM") as pz0pool,
            tc.tile_pool(name="pz1", bufs=1, space="PSUM") as pz1pool,
            tc.tile_pool(name="pz2", bufs=1, space="PSUM") as pz2pool,
            tc.tile_pool(name="ppo", bufs=1, space="PSUM") as popool,
        ):
            pzpools = [pz0pool, pz1pool, pz2pool]

            x_tiles = {}

            def load_x(k, split=False):
                b, j2 = divmod(k, NT // 2)
                xt = xpool.tile([C_IN, 2 * F], bf16, name="xt", tag="xt")
                if split:
                    nc.sync.dma_start(out=xt[:, 0:F],
                                      in_=x[b, :, j2 * 2 * F:j2 * 2 * F + F])
                    nc.sync.dma_start(out=xt[:, F:2 * F],
                                      in_=x[b, :, j2 * 2 * F + F:(j2 + 1) * 2 * F])
                else:
                    nc.sync.dma_start(out=xt[:],
                                      in_=x[b, :, j2 * 2 * F:(j2 + 1) * 2 * F])
                x_tiles[k] = xt

            # Latency-ordered prologue DMAs: first x pair + mm1 operands
            # first, so the PE can start immediately; mm3's table is only
            # needed ~10 rounds in.
            load_x(0, split=True)
            w1_t = cpool.tile([C_IN, HC], bf16, name="w1_t")
            nc.sync.dma_start(out=w1_t[:], in_=w1[:])
            b1_t = cpool.tile([128, 1], f32, name="b1_t")
            nc.sync.dma_start(out=b1_t[:], in_=b1d[:])
            w2_t = cpool.tile([128, 384], bf16, name="w2_t")
            nc.sync.dma_start(out=w2_t[:], in_=w2d[:])
            b2n_t = cpool.tile([128, 3], f32, name="b2n_t")
            nc.sync.dma_start(out=b2n_t[:], in_=b2n[:])
            b2p_t = cpool.tile([128, 3], f32, name="b2p_t")
            nc.sync.dma_start(out=b2p_t[:], in_=b2p[:])
            w3_t = cpool.tile([128, 3 * GS * 64], bf16, name="w3_t")
            nc.sync.dma_start(out=w3_t[:], in_=w3s[:])

            py_tiles = {}   # pair -> psum tile [128, F]
            y_tiles = {}    # pair -> sbuf tile [128, F] bf16
            pz_tiles = {}   # (pair, p) -> psum tile [128, 2F]
            z_tiles = {}    # (pair, p) -> sbuf tile [128, 2F] bf16

            def mm1(t):
                pk, parity = divmod(t, 2)
                if parity == 0:
                    py_tiles[pk] = pypool.tile([128, F], f32, name="py",
                                               tag="py")
                py = py_tiles[pk]
                c = 64 * parity
                nc.tensor.matmul(py[c:c + 64, :], w1_t[:, 0:HC],
                                 x_tiles[pk][:, parity * F:(parity + 1) * F],
                                 start=True, stop=True)

            def act1(pk):
                py = py_tiles.pop(pk)
                y2 = ypool.tile([128, F], bf16, name="y2", tag="y2")
                nc.scalar.activation(y2[:], py[:], AF.Relu,
                                     bias=b1_t[:, 0:1], scale=1.0)
                y_tiles[pk] = y2

            def quad(pk, p):
                y2 = y_tiles[pk]
                pz = pzpools[p].tile([128, 2 * F], f32, name=f"pz{p}",
                                     tag=f"pz{p}")
                pz_tiles[(pk, p)] = pz
                for parity in range(2):
                    r = 64 * parity
                    for half in range(2):
                        c = 64 * half
                        nc.tensor.matmul(
                            pz[c:c + 64, parity * F:(parity + 1) * F],
                            w2_t[r:r + 64, 128 * p + c:128 * p + c + 64],
                            y2[r:r + 64, :], start=True, stop=True)

            def zrelu(pk, p):
                pz = pz_tiles.pop((pk, p))
                if p == 2:
                    # PSUM is reachable only from ACT and DVE (no GPSIMD
                    # port, no DMA route). ACT: zp1+zp2; DVE: act1+zp0+epi.
                    z2 = z2pool.tile([128, 2 * F], bf16, name="z2", tag="z2")
                    if pk % 2 == 0:
                        nc.scalar.activation(z2[:], pz[:], AF.Relu,
                                             bias=b2p_t[:, 2:3], scale=1.0)
                    else:
                        nc.vector.tensor_scalar(z2[:], pz[:],
                                                b2p_t[:, 2:3], 0.0,
                                                ALU.add, ALU.max)
                    z_tiles[(pk, p)] = z2
                elif p == 1:
                    z1 = z1pool.tile([128, 2 * F], bf16, name="z1", tag="z1")
                    nc.scalar.activation(z1[:], pz[:], AF.Relu,
                                         bias=b2p_t[:, 1:2], scale=1.0)
                    z_tiles[(pk, p)] = z1
                else:
                    z0 = z0pool.tile([128, 2 * F], bf16, name="z0", tag="z0")
                    nc.vector.tensor_scalar(z0[:], pz[:], b2p_t[:, 0:1],
                                            0.0, ALU.add, ALU.max)
                    z_tiles[(pk, p)] = z0

            # po: ONE psum bank; even groups accumulate in partitions 0-63,
            # odd groups in 64-127, so two groups pipeline concurrently.
            po = popool.tile([128, F], f32, name="po", tag="po")

            # consume ACT-evicted z (p1) first each pair: DVE paces the
            # pipeline, so give its evictions extra rounds of slack before
            # mm3 needs them.
            P_ORDER = [1, 0, 2]

            def mm3_stream(g, t, idx):
                p = P_ORDER[idx]
                s = t % GS
                c = 64 * (g % 2)
                pk, parity = divmod(t, 2)
                z = z_tiles[(pk, p)]
                nc.tensor.matmul(
                    po[c:c + 64, :],
                    w3_t[:, (p * GS + s) * 64:(p * GS + s + 1) * 64],
                    z[:, parity * F:(parity + 1) * F],
                    start=(s == 0 and idx == 0),
                    stop=(s == GS - 1 and idx == 2),
                    skip_group_check=True)

            e_tiles = {}

            def epi(g):
                # DMA cannot read PSUM: evict through a DVE copy first.
                c = 64 * (g % 2)
                e = stpool.tile([128, F], f32, name="e", tag="e")
                nc.vector.tensor_copy(e[c:c + 48, :], po[c:c + 48, :])
                e_tiles[g] = e

            def out_dma(g):
                # one pair later than epi(g): by then the copy is done, so
                # this DMA never head-of-line-blocks the x loads on SP.
                c = 64 * (g % 2)
                tg = g * GS
                b, j0 = divmod(tg, NT)
                e = e_tiles.pop(g)
                dview = out[b, :, j0 * F:(j0 + GS) * F]
                dview = dview.rearrange("c (s f) -> s c f", s=GS)
                nc.sync.dma_start(out=dview, in_=e[c:c + 48, :])

            def mm3_lane(k, lane):
                # groups active at pair k: 2g+1 <= k <= 2g+4; even g -> L
                for g in range((k - 4 + 1) // 2, (k - 1) // 2 + 1):
                    if 0 <= g < NG and (g % 2 == 0) == (lane == 0):
                        return g, 4 * g + (k - 2 * g - 1)
                return None

            # HAM pre-warm: dummy matmuls on a memset scratch tile keep the
            # PE busy while the first x tiles land.
            scr = cpool.tile([C_IN, F], bf16, name="scr")
            nc.vector.memset(scr[:], 0.0)
            # Alternate two psum targets (py slot / po) so consecutive
            # dummies have no WAW chain and issue back-to-back; ~3.5us of
            # continuous PE activity trips the HAM un-throttle early.
            for i in range(8):
                if i % 2 == 0:
                    pd = pypool.tile([128, F], f32, name="pd", tag="py")
                    nc.tensor.matmul(pd[0:128, 0:256], scr[:, 0:128],
                                     scr[:, 0:256], start=True, stop=True)
                else:
                    nc.tensor.matmul(po[0:128, 0:256], scr[:, 0:128],
                                     scr[:, 0:256], start=True, stop=True)

            # Prologue: pair 0's mm1 + act1; bridge dummies cover the
            # act1 latency so the PE never idles cold.
            mm1(0)
            mm1(1)
            act1(0)
            for i in range(3):
                pd = pypool.tile([128, F], f32, name="pd", tag="py")
                nc.tensor.matmul(pd[0:128, 0:128], scr[:, 0:128],
                                 scr[:, 0:128], start=True, stop=True)
            for k in range(1, 6):
                load_x(k)

            # Main loop: pair k runs quads(k), mm1 of pair k+1, and the two
            # active mm3 group lanes; k extends past NPAIR to drain mm3.
            for k in range(0, NPAIR + 5):
                mm3L = mm3_lane(k, 0)
                mm3R = mm3_lane(k, 1)
                if k + 5 < NPAIR:
                    load_x(k + 5)
                # pipeline-fill pairs leave PE lanes empty; dense dummy
                # matmuls there keep the HAM activity window fed (po's odd
                # half is untouched until group 1 starts at pair 3).
                if k < 3:
                    for _ in range(3):
                        nc.tensor.matmul(po[64:128, 256:512],
                                         scr[:, 0:64], scr[:, 0:256],
                                         start=True, stop=True)
                # pairs 3-5 have no safe psum dummy target (both po halves
                # live), but dep-free LDWEIGHTS also count as PE activity:
                # bridge the fill-phase holes so HAM never re-throttles.
                if k < 6:
                    for _ in range(4):
                        nc.tensor.ldweights(scr[:, 0:128])
                # group g closed at the END of pair k-1 (= 2g+4); emit its
                # eviction copy FIRST so it isn't queued behind this pair's
                # z-evictions on DVE (it gates group g+2's first mm3).
                if k >= 5 and k % 2 == 1 and (k - 5) // 2 < NG:
                    epi((k - 5) // 2)
                if k >= 6 and k % 2 == 0 and (k - 6) // 2 < NG:
                    out_dma((k - 6) // 2)
                # r1
                if 2 * k + 2 < T:
                    mm1(2 * k + 2)
                if mm3R:
                    mm3_stream(mm3R[0], mm3R[1], 0)
                # r2
                if k < NPAIR:
                    quad(k, 0)
                    zrelu(k, 0)
                # r3
                if mm3L:
                    mm3_stream(mm3L[0], mm3L[1], 0)
                if 2 * k + 3 < T:
                    mm1(2 * k + 3)
                    act1(k + 1)
                # HAM keep-alive: a dep-free standalone LDWEIGHTS counts as
                # PE activity; it bridges the ~1us eviction-paced PE holes
                # so the MID window never sees the PE as idle.
                nc.tensor.ldweights(scr[:, 0:128])
                # r4
                if k < NPAIR:
                    quad(k, 1)
                    zrelu(k, 1)
                # r5
                if mm3L:
                    mm3_stream(mm3L[0], mm3L[1], 1)
                if mm3R:
                    mm3_stream(mm3R[0], mm3R[1], 1)
                # r6
                if k < NPAIR:
                    quad(k, 2)
                    zrelu(k, 2)
                nc.tensor.ldweights(scr[:, 0:128])
                # r7
                if mm3L:
                    mm3_stream(mm3L[0], mm3L[1], 2)
                if mm3R:
                    mm3_stream(mm3R[0], mm3R[1], 2)
    _split_waits(nc)
    return nc


def _split_waits(nc, cap=1):
    """This container's walrus build rejects instructions carrying more than
    a small number of sync waits (matmuls: just one). Move excess waits onto
    single-wait NoOp carriers inserted before the instruction on the same
    engine."""
    import concourse.mybir as mybir

    k = 0
    for func in nc.m.functions:
        for bb in func.blocks:
            insts = bb.instructions
            out_insts = []
            changed = False
            for inst in insts:
                si = inst.sync_info
                waits = list(si.on_wait) if si and si.on_wait else []
                if len(waits) > cap:
                    for w in waits[:-cap]:
                        d = mybir.InstNoOp(name=f"I-sw{k}", ins=[], outs=[])
                        k += 1
                        d.engine = inst.engine
                        d.sync_info = mybir.SyncInfo(on_wait=[w], on_update=[])
                        nc.register_instruction(d)
                        out_insts.append(d)
                    inst.sync_info = mybir.SyncInfo(
                        on_wait=waits[-cap:],
                        on_update=list(si.on_update) if si.on_update else [])
                    changed = True
                out_insts.append(inst)
            if changed:
                bb.instructions = out_insts


def _get_program():
    if "nc" not in _CACHE:
        _CACHE["nc"] = _build_program()
    return _CACHE["nc"]


def _prep_weights(d):
    """Fold BN into conv weights/biases; pack stationary matrices (bf16)."""
    import ml_dtypes
    bf16 = ml_dtypes.bfloat16
    f8 = np.float64

    def g(name):
        return np.asarray(d[name], dtype=f8)

    # shared conv + BN
    s1 = g("sh_g") / np.sqrt(g("sh_var") + EPS)                     # [64]
    W1e = g("sh_w") * s1[:, None]                                   # [64,128]
    b1e = g("sh_b") * s1 + g("sh_beta") - g("sh_mean") * s1         # [64]
    w1 = W1e.T.copy()                                               # [128,64]
    b1d = np.concatenate([b1e, b1e])[:, None]                       # [128,1]

    # head first layers + BN: pair p holds heads 2p (z rows 0-63) and 2p+1
    # (rows 64-127); stationary duplicated on rows 64-127 so odd tiles
    # (row-position 64) can use it.
    s2 = g("head_g1") / np.sqrt(g("head_var1") + EPS)               # [6,64]
    W2e = g("head_w1") * s2[:, :, None]                             # [6,64,64]
    b2e = g("head_b1") * s2 + g("head_beta1") - g("head_mean1") * s2  # [6,64]
    w2d = np.zeros((128, 384), f8)
    b2 = np.zeros((128, 3), f8)
    for p in range(3):
        blk = np.zeros((64, 128), f8)
        blk[:, 0:64] = W2e[2 * p].T
        blk[:, 64:128] = W2e[2 * p + 1].T
        w2d[0:64, 128 * p:128 * (p + 1)] = blk
        w2d[64:128, 128 * p:128 * (p + 1)] = blk
        b2[0:64, p] = b2e[2 * p]
        b2[64:128, p] = b2e[2 * p + 1]

    # final convs: per (p, slot s) a [128, 64] block, live cols 12s..12s+12
    names = ["hm", "reg", "height", "dim", "rot", "iou"]
    Wf = [g(n + "_w") for n in names]
    bf = [g(n + "_b") for n in names]
    w3s = np.zeros((128, 3 * GS * 64), f8)
    b3 = np.zeros((12,), f8)
    for p in range(3):
        ha, hb = 2 * p, 2 * p + 1
        ca, cb = HEAD_OUT[ha], HEAD_OUT[hb]
        off = PAIR_OFF[p]
        blk = np.zeros((128, 12), f8)
        blk[0:64, off:off + ca] = Wf[ha].T
        blk[64:128, off + ca:off + ca + cb] = Wf[hb].T
        b3[off:off + ca] = bf[ha]
        b3[off + ca:off + ca + cb] = bf[hb]
        for s in range(GS):
            c0 = (p * GS + s) * 64
            w3s[:, c0 + 12 * s:c0 + 12 * s + 12] = blk

    return ({"w1": w1.astype(bf16), "b1d": b1d.astype(np.float32),
             "w2d": w2d.astype(bf16),
             "b2n": (-b2).astype(np.float32), "b2p": b2.astype(np.float32),
             "w3s": w3s.astype(bf16)},
            b3.astype(np.float32))


def _ensure_ntff_hook():
    """Install the antenv.axon_hooks NTFF-profile shim if the container's
    antenv package lacks it (profiling only; never used in grading runs)."""
    try:
        from antenv.axon_hooks import get_axon_ntff_profile_hook  # noqa: F401
        return True
    except ImportError:
        pass
    import contextlib
    import ctypes
    import sys as _sys
    import types

    so_path = "/opt/axon/libaxon_pjrt.so"
    if not os.path.exists(so_path):
        return False
    lib = ctypes.CDLL(so_path)
    if not hasattr(lib, "axon_start_nrt_profile"):
        return False
    lib.axon_start_nrt_profile.argtypes = [ctypes.POINTER(ctypes.c_int64),
                                           ctypes.c_size_t]
    lib.axon_start_nrt_profile.restype = ctypes.c_int64
    lib.axon_stop_nrt_profile.argtypes = [ctypes.c_char_p]
    lib.axon_stop_nrt_profile.restype = ctypes.c_int64

    @contextlib.contextmanager
    def _hook(output_dir, device_ids):
        import jax
        jax.devices()
        if device_ids:
            ids = (ctypes.c_int64 * len(device_ids))(*device_ids)
            rc = lib.axon_start_nrt_profile(ids, len(device_ids))
        else:
            rc = lib.axon_start_nrt_profile(None, 0)
        if rc != 0:
            raise RuntimeError(f"axon_start_nrt_profile rc={rc}")
        try:
            yield
        finally:
            n = lib.axon_stop_nrt_profile(str(output_dir).encode())
            print(f"profile: {n} file(s) written to {output_dir}",
                  file=sys.stderr)

    import antenv
    mod = types.ModuleType("antenv.axon_hooks")
    mod.get_axon_ntff_profile_hook = lambda: _hook
    mod.set_axon_ntff_profile_hook = lambda h: None
    _sys.modules["antenv.axon_hooks"] = mod
    antenv.axon_hooks = mod
    return True


def kernel(**inputs):
    global LAST_RESULTS, LAST_EXEC_NS
    import ml_dtypes
    from concourse.bass_utils import run_bass_kernel_spmd

    inputs = {k: np.asarray(v) for k, v in inputs.items()}
    weights, b3 = _prep_weights(inputs)

    ct = np.asarray(inputs["ct_feat"], dtype=np.float32)
    xs = ct.astype(ml_dtypes.bfloat16).reshape(NCORES, BC, C_IN, N)

    in_maps = [dict(weights, x=np.ascontiguousarray(xs[i]))
               for i in range(NCORES)]

    nc = _get_program()
    trace = bool(int(os.environ.get("CK_PROFILE", "0")))
    if trace:
        trace = _ensure_ntff_hook()
    res = run_bass_kernel_spmd(nc, in_maps, list(range(NCORES)), trace=trace)
    LAST_RESULTS = res
    LAST_EXEC_NS = res.exec_time_ns

    out = np.concatenate([np.asarray(res.results[i]["out"])
                          for i in range(NCORES)], axis=0)
    return (out + b3[None, :, None]).astype(np.float32)


# revision 45
# speedup vs baseline: 1.0135x; 1.0135x over previous
"""Trainium2 Bass kernel: CenterHeadIoU 1x1-conv stack (v2: PE quadrant packing).

Computes, for x = ct_feat [B=32, C=128, N=8192]:
  y = relu(bn(sh_w @ x))                       [B, 64, N]
  z_h = relu(bn_h(head_w1[h] @ y)), h=0..5     [B, 64, N] each
  out = concat_h(head_final_w[h] @ z_h) + b    [B, 12, N]

Sharding: data-parallel over batch, 4 batches per core on 8 cores; weights
replicated. BN folded into weights on host; matmul operands bf16. The final
per-channel bias b3 is added on the HOST after the gather (frees a PSUM
epilogue op per group).

v2 exploits PE sub-array concurrency (tile_position): matmuls whose
(row-group, col-group) rectangles are disjoint execute CONCURRENTLY
(HW-probed: 4-quadrant K=64/M=64 packs at 61ns/MM vs 216 serial; K=128/M=64
column pairs at 127ns/MM). Per 2-tile pair, 7 rounds of ~512 PE cycles:

  r1: mm1(u')  @cols 0-63   || mm3[gR]  @cols 64-127
  r2: QUAD mm2-p0: 4x [K=64,M=64] in the 4 array quadrants
  r3: mm3[gL]  @cols 0-63   || mm1(u'+1) @cols 64-127
  r4: QUAD mm2-p1
  r5: mm3[gL]               || mm3[gR]
  r6: QUAD mm2-p2
  r7: mm3[gL]               || mm3[gR]

mm1 writes the pair's pre-y into ONE psum bank vertically (tile u rows 0-63
@col0, tile u+1 rows 64-127 @col64), so act1 is one [128,512] ACT op per
pair. mm2 runs as 4 quadrant halves per p (y(u) feeds row-pos 0, y(u+1)
row-pos 64, via a duplicated stationary) writing one [128,1024] psum tile.
mm3 accumulates 4 tiles x 3 p-blocks (12 streams, K=128 M=64, live cols
12s..12s+12) into a po half-bank; EVEN groups use cols/partitions 0-63, ODD
64-127, so two groups pipeline in ONE bank and pair up in rounds r5/r7.
Group g's streams run at pairs 2g+1..2g+4, one tile per pair. Output is
DMA'd straight from PSUM (12-partition stripes), bias added on host.

PSUM: py 1 + pz 3x2 + po 1 = 8 banks exactly.

Elementwise (z eviction = relu+bias+bf16, psum->sbuf) is spread over the
only two PSUM-capable engines plus GPSIMD via a staging bounce:
  ACT: act1 [128,512] + z-p1 [128,1024]          (~1.68us/pair)
  DVE: z-p0 [128,1024] tensor_scalar(max -b, add b)  (~1.24us/pair)
  GPS: z-p2 2x [128,512] from an SBUF f32 stage (PSUM->SBUF DMA bounce;
       GPSIMD has no PSUM port)                   (~1.68us/pair)
relu(z+b) == max(z,-b)+b lets tensor_scalar apply the bias without ACT.

The PE starts clock-gated at 1.2 GHz (HAM) and reaches 2.4 GHz only after
~3.4us of sustained activity; the prologue front-loads latency-ordered DMAs
and pre-warms the PE with dummy matmuls on a memset scratch tile.

A post-pass moves multi-wait sync conditions onto single-wait NoOp carriers
(this walrus build caps sync waits per instruction).
"""

import os
import sys
import numpy as np

B, C_IN, N, HC = 32, 128, 8192, 64
NCORES = 8
BC = B // NCORES            # batches per core
F = 512                     # free-dim tile = one fp32 PSUM bank
NT = N // F                 # tiles per batch
T = BC * NT                 # tiles per core
NPAIR = T // 2              # 2-tile pairs per core
GS = 4                      # tiles per mm3 output group
NG = T // GS                # groups per core
EPS = 1e-5
HEAD_OUT = [3, 2, 1, 3, 2, 1]        # hm, reg, height, dim, rot, iou
PAIR_OFF = [0, 5, 9]                 # channel offset of pair p in the 12-ch out

_CACHE = {}
LAST_RESULTS = None
LAST_EXEC_NS = None


def _build_program():
    import concourse.bass as bass
    import concourse.mybir as mybir
    import concourse.tile as tile

    f32 = mybir.dt.float32
    bf16 = mybir.dt.bfloat16
    AF = mybir.ActivationFunctionType
    ALU = mybir.AluOpType

    nc = bass.Bass("TRN2", target_bir_lowering=False, debug=False,
                   num_devices=NCORES)

    x = nc.dram_tensor("x", [BC, C_IN, N], bf16, kind="ExternalInput").ap()
    w1 = nc.dram_tensor("w1", [C_IN, HC], bf16, kind="ExternalInput").ap()
    b1d = nc.dram_tensor("b1d", [128, 1], f32, kind="ExternalInput").ap()
    w2d = nc.dram_tensor("w2d", [128, 384], bf16, kind="ExternalInput").ap()
    b2n = nc.dram_tensor("b2n", [128, 3], f32, kind="ExternalInput").ap()
    b2p = nc.dram_tensor("b2p", [128, 3], f32, kind="ExternalInput").ap()
    w3s = nc.dram_tensor("w3s", [128, 3 * GS * 64], bf16,
                         kind="ExternalInput").ap()
    out = nc.dram_tensor("out", [BC, 12, N], f32, kind="ExternalOutput").ap()

    with tile.TileContext(nc) as tc:
        with (
            tc.tile_pool(name="consts", bufs=1) as cpool,
            tc.tile_pool(name="xin", bufs=7) as xpool,
            tc.tile_pool(name="ysb", bufs=5) as ypool,
            tc.tile_pool(name="z0sb", bufs=5) as z0pool,
            tc.tile_pool(name="z1sb", bufs=5) as z1pool,
            tc.tile_pool(name="z2sb", bufs=5) as z2pool,
            tc.tile_pool(name="zst", bufs=4) as stpool,
            tc.tile_pool(name="ppy", bufs=1, space="PSUM") as pypool,
            tc.tile_pool(name="pz0", bufs=1, space="PSUM") as pz0pool,
            tc.tile_pool(name="pz1", bufs=1, space="PSUM") as pz1pool,
            tc.tile_pool(name="pz2", bufs=1, space="PSUM") as pz2pool,
            tc.tile_pool(name="ppo", bufs=1, space="PSUM") as popool,
        ):
            pzpools = [pz0pool, pz1pool, pz2pool]

            x_tiles = {}

            def load_x(k, split=False):
                b, j2 = divmod(k, NT // 2)
                xt = xpool.tile([C_IN, 2 * F], bf16, name="xt", tag="xt")
                if split:
                    nc.sync.dma_start(out=xt[:, 0:F],
                                      in_=x[b, :, j2 * 2 * F:j2 * 2 * F + F])
                    nc.sync.dma_start(out=xt[:, F:2 * F],
                                      in_=x[b, :, j2 * 2 * F + F:(j2 + 1) * 2 * F])
                else:
                    nc.sync.dma_start(out=xt[:],
                                      in_=x[b, :, j2 * 2 * F:(j2 + 1) * 2 * F])
                x_tiles[k] = xt

            # Latency-ordered prologue DMAs: first x pair + mm1 operands
            # first, so the PE can start immediately; mm3's table is only
            # needed ~10 rounds in.
            load_x(0, split=True)
            w1_t = cpool.tile([C_IN, HC], bf16, name="w1_t")
            nc.sync.dma_start(out=w1_t[:], in_=w1[:])
            b1_t = cpool.tile([128, 1], f32, name="b1_t")
            nc.sync.dma_start(out=b1_t[:], in_=b1d[:])
            w2_t = cpool.tile([128, 384], bf16, name="w2_t")
            nc.sync.dma_start(out=w2_t[:], in_=w2d[:])
            b2n_t = cpool.tile([128, 3], f32, name="b2n_t")
            nc.sync.dma_start(out=b2n_t[:], in_=b2n[:])
            b2p_t = cpool.tile([128, 3], f32, name="b2p_t")
            nc.sync.dma_start(out=b2p_t[:], in_=b2p[:])
            w3_t = cpool.tile([128, 3 * GS * 64], bf16, name="w3_t")
            nc.sync.dma_start(out=w3_t[:], in_=w3s[:])

            py_tiles = {}   # pair -> psum tile [128, F]
            y_tiles = {}    # pair -> sbuf tile [128, F] bf16
            pz_tiles = {}   # (pair, p) -> psum tile [128, 2F]
            z_tiles = {}    # (pair, p) -> sbuf tile [128, 2F] bf16

            def mm1(t):
                pk, parity = divmod(t, 2)
                if parity == 0:
                    py_tiles[pk] = pypool.tile([128, F], f32, name="py",
                                               tag="py")
                py = py_tiles[pk]
                c = 64 * parity
                nc.tensor.matmul(py[c:c + 64, :], w1_t[:, 0:HC],
                                 x_tiles[pk][:, parity * F:(parity + 1) * F],
                                 start=True, stop=True)

            def act1(pk):
                py = py_tiles.pop(pk)
                y2 = ypool.tile([128, F], bf16, name="y2", tag="y2")
                nc.scalar.activation(y2[:], py[:], AF.Relu,
                                     bias=b1_t[:, 0:1], scale=1.0)
                y_tiles[pk] = y2

            def quad(pk, p):
                y2 = y_tiles[pk]
                pz = pzpools[p].tile([128, 2 * F], f32, name=f"pz{p}",
                                     tag=f"pz{p}")
                pz_tiles[(pk, p)] = pz
                for parity in range(2):
                    r = 64 * parity
                    for half in range(2):
                        c = 64 * half
                        nc.tensor.matmul(
                            pz[c:c + 64, parity * F:(parity + 1) * F],
                            w2_t[r:r + 64, 128 * p + c:128 * p + c + 64],
                            y2[r:r + 64, :], start=True, stop=True)

            def zrelu(pk, p):
                pz = pz_tiles.pop((pk, p))
                if p == 2:
                    # PSUM is reachable only from ACT and DVE (no GPSIMD
                    # port, no DMA route). ACT: zp1+zp2; DVE: act1+zp0+epi.
                    z2 = z2pool.tile([128, 2 * F], bf16, name="z2", tag="z2")
                    if pk % 2 == 0:
                        nc.scalar.activation(z2[:], pz[:], AF.Relu,
                                             bias=b2p_t[:, 2:3], scale=1.0)
                    else:
                        nc.vector.tensor_scalar(z2[:], pz[:],
                                                b2p_t[:, 2:3], 0.0,
                                                ALU.add, ALU.max)
                    z_tiles[(pk, p)] = z2
                elif p == 1:
                    z1 = z1pool.tile([128, 2 * F], bf16, name="z1", tag="z1")
                    nc.scalar.activation(z1[:], pz[:], AF.Relu,
                                         bias=b2p_t[:, 1:2], scale=1.0)
                    z_tiles[(pk, p)] = z1
                else:
                    z0 = z0pool.tile([128, 2 * F], bf16, name="z0", tag="z0")
                    nc.vector.tensor_scalar(z0[:], pz[:], b2p_t[:, 0:1],
                                            0.0, ALU.add, ALU.max)
                    z_tiles[(pk, p)] = z0

            # po: ONE psum bank; even groups accumulate in partitions 0-63,
            # odd groups in 64-127, so two groups pipeline concurrently.
            po = popool.tile([128, F], f32, name="po", tag="po")

            # consume ACT-evicted z (p1) first each pair: DVE paces the
            # pipeline, so give its evictions extra rounds of slack before
            # mm3 needs them.
            P_ORDER = [1, 0, 2]

            def mm3_stream(g, t, idx):
                p = P_ORDER[idx]
                s = t % GS
                c = 64 * (g % 2)
                pk, parity = divmod(t, 2)
                z = z_tiles[(pk, p)]
                nc.tensor.matmul(
                    po[c:c + 64, :],
                    w3_t[:, (p * GS + s) * 64:(p * GS + s + 1) * 64],
                    z[:, parity * F:(parity + 1) * F],
                    start=(s == 0 and idx == 0),
                    stop=(s == GS - 1 and idx == 2),
                    skip_group_check=True)

            e_tiles = {}

            def epi(g):
                # DMA cannot read PSUM: evict through a DVE copy first.
                c = 64 * (g % 2)
                e = stpool.tile([128, F], f32, name="e", tag="e")
                nc.vector.tensor_copy(e[c:c + 48, :], po[c:c + 48, :])
                e_tiles[g] = e

            def out_dma(g):
                # one pair later than epi(g): by then the copy is done, so
                # this DMA never head-of-line-blocks the x loads on SP.
                c = 64 * (g % 2)
                tg = g * GS
                b, j0 = divmod(tg, NT)
                e = e_tiles.pop(g)
                dview = out[b, :, j0 * F:(j0 + GS) * F]
                dview = dview.rearrange("c (s f) -> s c f", s=GS)
                nc.sync.dma_start(out=dview, in_=e[c:c + 48, :])

            def mm3_lane(k, lane):
                # groups active at pair k: 2g+1 <= k <= 2g+4; even g -> L
                for g in range((k - 4 + 1) // 2, (k - 1) // 2 + 1):
                    if 0 <= g < NG and (g % 2 == 0) == (lane == 0):
                        return g, 4 * g + (k - 2 * g - 1)
                return None

            # HAM pre-warm: dummy matmuls on a memset scratch tile keep the
            # PE busy while the first x tiles land.
            scr = cpool.tile([C_IN, F], bf16, name="scr")
            nc.vector.memset(scr[:], 0.0)
            # Alternate two psum targets (py slot / po) so consecutive
            # dummies have no WAW chain and issue back-to-back; ~3.5us of
            # continuous PE activity trips the HAM un-throttle early.
            for i in range(8):
                if i % 2 == 0:
                    pd = pypool.tile([128, F], f32, name="pd", tag="py")
                    nc.tensor.matmul(pd[0:128, 0:256], scr[:, 0:128],
                                     scr[:, 0:256], start=True, stop=True)
                else:
                    nc.tensor.matmul(po[0:128, 0:256], scr[:, 0:128],
                                     scr[:, 0:256], start=True, stop=True)

            # Prologue: pair 0's mm1 + act1; bridge dummies cover the
            # act1 latency so the PE never idles cold.
            mm1(0)
            mm1(1)
            act1(0)
            for i in range(3):
                pd = pypool.tile([128, F], f32, name="pd", tag="py")
                nc.tensor.matmul(pd[0:128, 0:128], scr[:, 0:128],
                                 scr[:, 0:128], start=True, stop=True)
            for k in range(1, 6):
                load_x(k)

            # Main loop: pair k runs quads(k), mm1 of pair k+1, and the two
            # active mm3 group lanes; k extends past NPAIR to drain mm3.
            for k in range(0, NPAIR + 5):
                mm3L = mm3_lane(k, 0)
                mm3R = mm3_lane(k, 1)
                if k + 5 < NPAIR:
                    load_x(k + 5)
                # pipeline-fill pairs leave PE lanes empty; dense dummy
                # matmuls there keep the HAM activity window fed (po's odd
                # half is untouched until group 1 starts at pair 3).
                if k < 3:
                    for _ in range(3):
                        nc.tensor.matmul(po[64:128, 256:512],
                                         scr[:, 0:64], scr[:, 0:256],
                                         start=True, stop=True)
                # group g closed at the END of pair k-1 (= 2g+4); emit its
                # eviction copy FIRST so it isn't queued behind this pair's
                # z-evictions on DVE (it gates group g+2's first mm3).
                if k >= 5 and k % 2 == 1 and (k - 5) // 2 < NG:
                    epi((k - 5) // 2)
                if k >= 6 and k % 2 == 0 and (k - 6) // 2 < NG:
                    out_dma((k - 6) // 2)
                # r1
                if 2 * k + 2 < T:
                    mm1(2 * k + 2)
                if mm3R:
                    mm3_stream(mm3R[0], mm3R[1], 0)
                # r2
                if k < NPAIR:
                    quad(k, 0)
                    zrelu(k, 0)
                # r3
                if mm3L:
                    mm3_stream(mm3L[0], mm3L[1], 0)
                if 2 * k + 3 < T:
                    mm1(2 * k + 3)
                    act1(k + 1)
                # HAM keep-alive: a dep-free standalone LDWEIGHTS counts as
                # PE activity; it bridges the ~1us eviction-paced PE holes
                # so the MID window never sees the PE as idle.
                nc.tensor.ldweights(scr[:, 0:128])
                # r4
                if k < NPAIR:
                    quad(k, 1)
                    zrelu(k, 1)
                # r5
                if mm3L:
                    mm3_stream(mm3L[0], mm3L[1], 1)
                if mm3R:
                    mm3_stream(mm3R[0], mm3R[1], 1)
                # r6
                if k < NPAIR:
                    quad(k, 2)
                    zrelu(k, 2)
                nc.tensor.ldweights(scr[:, 0:128])
                # r7
                if mm3L:
                    mm3_stream(mm3L[0], mm3L[1], 2)
                if mm3R:
                    mm3_stream(mm3R[0], mm3R[1], 2)
    _split_waits(nc)
    return nc


def _split_waits(nc, cap=1):
    """This container's walrus build rejects instructions carrying more than
    a small number of sync waits (matmuls: just one). Move excess waits onto
    single-wait NoOp carriers inserted before the instruction on the same
    engine."""
    import concourse.mybir as mybir

    k = 0
    for func in nc.m.functions:
        for bb in func.blocks:
            insts = bb.instructions
            out_insts = []
            changed = False
            for inst in insts:
                si = inst.sync_info
                waits = list(si.on_wait) if si and si.on_wait else []
                if len(waits) > cap:
                    for w in waits[:-cap]:
                        d = mybir.InstNoOp(name=f"I-sw{k}", ins=[], outs=[])
                        k += 1
                        d.engine = inst.engine
                        d.sync_info = mybir.SyncInfo(on_wait=[w], on_update=[])
                        nc.register_instruction(d)
                        out_insts.append(d)
                    inst.sync_info = mybir.SyncInfo(
                        on_wait=waits[-cap:],
                        on_update=list(si.on_update) if si.on_update else [])
                    changed = True
                out_insts.append(inst)
            if changed:
                bb.instructions = out_insts


def _get_program():
    if "nc" not in _CACHE:
        _CACHE["nc"] = _build_program()
    return _CACHE["nc"]


def _prep_weights(d):
    """Fold BN into conv weights/biases; pack stationary matrices (bf16)."""
    import ml_dtypes
    bf16 = ml_dtypes.bfloat16
    f8 = np.float64

    def g(name):
        return np.asarray(d[name], dtype=f8)

    # shared conv + BN
    s1 = g("sh_g") / np.sqrt(g("sh_var") + EPS)                     # [64]
    W1e = g("sh_w") * s1[:, None]                                   # [64,128]
    b1e = g("sh_b") * s1 + g("sh_beta") - g("sh_mean") * s1         # [64]
    w1 = W1e.T.copy()                                               # [128,64]
    b1d = np.concatenate([b1e, b1e])[:, None]                       # [128,1]

    # head first layers + BN: pair p holds heads 2p (z rows 0-63) and 2p+1
    # (rows 64-127); stationary duplicated on rows 64-127 so odd tiles
    # (row-position 64) can use it.
    s2 = g("head_g1") / np.sqrt(g("head_var1") + EPS)               # [6,64]
    W2e = g("head_w1") * s2[:, :, None]                             # [6,64,64]
    b2e = g("head_b1") * s2 + g("head_beta1") - g("head_mean1") * s2  # [6,64]
    w2d = np.zeros((128, 384), f8)
    b2 = np.zeros((128, 3), f8)
    for p in range(3):
        blk = np.zeros((64, 128), f8)
        blk[:, 0:64] = W2e[2 * p].T
        blk[:, 64:128] = W2e[2 * p + 1].T
        w2d[0:64, 128 * p:128 * (p + 1)] = blk
        w2d[64:128, 128 * p:128 * (p + 1)] = blk
        b2[0:64, p] = b2e[2 * p]
        b2[64:128, p] = b2e[2 * p + 1]

    # final convs: per (p, slot s) a [128, 64] block, live cols 12s..12s+12
    names = ["hm", "reg", "height", "dim", "rot", "iou"]
    Wf = [g(n + "_w") for n in names]
    bf = [g(n + "_b") for n in names]
    w3s = np.zeros((128, 3 * GS * 64), f8)
    b3 = np.zeros((12,), f8)
    for p in range(3):
        ha, hb = 2 * p, 2 * p + 1
        ca, cb = HEAD_OUT[ha], HEAD_OUT[hb]
        off = PAIR_OFF[p]
        blk = np.zeros((128, 12), f8)
        blk[0:64, off:off + ca] = Wf[ha].T
        blk[64:128, off + ca:off + ca + cb] = Wf[hb].T
        b3[off:off + ca] = bf[ha]
        b3[off + ca:off + ca + cb] = bf[hb]
        for s in range(GS):
            c0 = (p * GS + s) * 64
            w3s[:, c0 + 12 * s:c0 + 12 * s + 12] = blk

    return ({"w1": w1.astype(bf16), "b1d": b1d.astype(np.float32),
             "w2d": w2d.astype(bf16),
             "b2n": (-b2).astype(np.float32), "b2p": b2.astype(np.float32),
             "w3s": w3s.astype(bf16)},
            b3.astype(np.float32))


def _ensure_ntff_hook():
    """Install the antenv.axon_hooks NTFF-profile shim if the container's
    antenv package lacks it (profiling only; never used in grading runs)."""
    try:
        from antenv.axon_hooks import get_axon_ntff_profile_hook  # noqa: F401
        return True
    except ImportError:
        pass
    import contextlib
    import ctypes
    import sys as _sys
    import types

    so_path = "/opt/axon/libaxon_pjrt.so"
    if not os.path.exists(so_path):
        return False
    lib = ctypes.CDLL(so_path)
    if not hasattr(lib, "axon_start_nrt_profile"):
        return False
    lib.axon_start_nrt_profile.argtypes = [ctypes.POINTER(ctypes.c_int64),
                                           ctypes.c_size_t]
    lib.axon_start_nrt_profile.restype = ctypes.c_int64
    lib.axon_stop_nrt_profile.argtypes = [ctypes.c_char_p]
    lib.axon_stop_nrt_profile.restype = ctypes.c_int64

    @contextlib.contextmanager
    def _hook(output_dir, device_ids):
        import jax
        jax.devices()
        if device_ids:
            ids = (ctypes.c_int64 * len(device_ids))(*device_ids)
            rc = lib.axon_start_nrt_profile(ids, len(device_ids))
        else:
            rc = lib.axon_start_nrt_profile(None, 0)
        if rc != 0:
            raise RuntimeError(f"axon_start_nrt_profile rc={rc}")
        try:
            yield
        finally:
            n = lib.axon_stop_nrt_profile(str(output_dir).encode())
            print(f"profile: {n} file(s) written to {output_dir}",
                  file=sys.stderr)

    import antenv
    mod = types.ModuleType("antenv.axon_hooks")
    mod.get_axon_ntff_profile_hook = lambda: _hook
    mod.set_axon_ntff_profile_hook = lambda h: None
    _sys.modules["antenv.axon_hooks"] = mod
    antenv.axon_hooks = mod
    return True


def kernel(**inputs):
    global LAST_RESULTS, LAST_EXEC_NS
    import ml_dtypes
    from concourse.bass_utils import run_bass_kernel_spmd

    inputs = {k: np.asarray(v) for k, v in inputs.items()}
    weights, b3 = _prep_weights(inputs)

    ct = np.asarray(inputs["ct_feat"], dtype=np.float32)
    xs = ct.astype(ml_dtypes.bfloat16).reshape(NCORES, BC, C_IN, N)

    in_maps = [dict(weights, x=np.ascontiguousarray(xs[i]))
               for i in range(NCORES)]

    nc = _get_program()
    trace = bool(int(os.environ.get("CK_PROFILE", "0")))
    if trace:
        trace = _ensure_ntff_hook()
    res = run_bass_kernel_spmd(nc, in_maps, list(range(NCORES)), trace=trace)
    LAST_RESULTS = res
    LAST_EXEC_NS = res.exec_time_ns

    out = np.concatenate([np.asarray(res.results[i]["out"])
                          for i in range(NCORES)], axis=0)
    return (out + b3[None, :, None]).astype(np.float32)
